# revision 1
# baseline (speedup 1.0000x reference)
"""Trainium2 Bass kernel for nn_DecoderLayer_45174466020042 (B=2, S=2048, H=4096).

Tensor-parallel decoder layer on 8 NeuronCores: core c owns heads 4c..4c+4 and
the matching fc1/fc2 column/row slices; LayerNorm is folded algebraically into
the qkv/fc1 weights (x_hat = x*rstd plus two extra contraction rows carrying
-mu*rstd and the bias constants); all matmuls run in float32r (TF32-like,
full PE rate, ~11-bit mantissa). The host transposes activations to
feature-major, pre-tiles weights, and sums the 8 partial outputs.
"""
import sys

sys.path.insert(0, '/opt/trn_rl_repo')

import numpy as np
import concourse.bass as bass
import concourse.bacc as bacc
import concourse.tile as tile
from concourse import mybir
from concourse.bass_utils import run_bass_kernel_spmd

f32r = mybir.dt.float32r
f32 = mybir.dt.float32
MULT = mybir.AluOpType.mult
ADD = mybir.AluOpType.add
SUB = mybir.AluOpType.subtract
AF = mybir.ActivationFunctionType

B, S, H = 2, 2048, 4096
NH, HD = 32, 128
RD, HALF = 64, 32
EPS = 1e-5
SCALE = HD ** -0.5
ROPE_BASE = 10000.0
T = B * S                 # 4096 tokens
NKH = H // 128            # 32 k-tiles over H
TC = 512                  # token chunk
NCH = T // TC             # 8 chunks
NPAIR = NCH // 2          # 4 chunk pairs
HPC = NH // 8             # 4 heads per core
NMQ = 3 * HPC             # 12 qkv m-tiles per core
NMF1 = 4 * H // 8 // 128  # 16 fc1 m-tiles per core
NMO = H // 128            # 32 output m-tiles
NKF2 = NMF1               # 16 fc2 k-tiles per core
NJT = S // 128            # 16 j-tiles per (b, h)
NIC = S // TC             # 4 i-chunks per (b, h)
JPC = TC // 128           # 4 j-tiles per i-chunk width
MASKV = -600.0            # additive pre-scale mask; exp(MASKV*SCALE) ~ 1e-23

_cache = {}


def _build_program():
    nc = bacc.Bacc("TRN2", target_bir_lowering=False, debug=False)

    xd = nc.dram_tensor("x", [128, NKH, T], f32r, kind="ExternalInput")
    wqkv = nc.dram_tensor("wqkv", [NMQ, 128, NKH * 128], f32r, kind="ExternalInput")
    eqkv = nc.dram_tensor("eqkv", [2, NMQ * 128], f32r, kind="ExternalInput")
    wfc1 = nc.dram_tensor("wfc1", [NMF1, 128, NKH * 128], f32r, kind="ExternalInput")
    efc1 = nc.dram_tensor("efc1", [2, NMF1 * 128], f32r, kind="ExternalInput")
    wfc2 = nc.dram_tensor("wfc2", [NMO, 128, NKF2 * 128], f32r, kind="ExternalInput")
    wdns = nc.dram_tensor("wdns", [NMO, 128, HPC * 128], f32r, kind="ExternalInput")
    cosd = nc.dram_tensor("cos", [HALF, B, S], f32r, kind="ExternalInput")
    sind = nc.dram_tensor("sin", [HALF, B, S], f32r, kind="ExternalInput")
    mask4 = nc.dram_tensor("mask4", [128, 4, TC], f32, kind="ExternalInput")
    identd = nc.dram_tensor("ident", [128, 128], f32r, kind="ExternalInput")
    onescd = nc.dram_tensor("onesc", [128, 1], f32r, kind="ExternalInput")
    onesrd = nc.dram_tensor("onesr", [1, 128], f32r, kind="ExternalInput")
    ones512d = nc.dram_tensor("ones512", [1, TC], f32r, kind="ExternalInput")
    outd = nc.dram_tensor("out", [128, NMO, T], f32, kind="ExternalOutput")

    # internal DRAM spills
    qs = nc.dram_tensor("qs", [HPC, 128, T], f32r)
    ks = nc.dram_tensor("ks", [HPC, 128, T], f32r)
    vs = nc.dram_tensor("vs", [HPC, 128, T], f32r)
    attns = nc.dram_tensor("attns", [HPC, 128, T], f32r)
    statsd = nc.dram_tensor("statsd", [2, T], f32r)  # row0 rstd, row1 s

    with tile.TileContext(nc) as tc:
        with tc.tile_pool(name="gl", bufs=1) as gl:
            onesc_t = gl.tile([128, 1], f32r, tag="onesc")
            nc.sync.dma_start(onesc_t[:], onescd[:])
            onesr_t = gl.tile([1, 128], f32r, tag="onesr")
            nc.sync.dma_start(onesr_t[:], onesrd[:])
            ones512_t = gl.tile([1, TC], f32r, tag="ones512")
            nc.sync.dma_start(ones512_t[:], ones512d[:])

            def ln_chunk_scale(pool, psx, xb, hsl, ch, xe, rstd_r):
                """Replicate rstd over 128 partitions, scale x in place, and
                finish the xe (extra contraction rows) tile."""
                nc.sync.dma_start(xe[1:2, :], ones512d[:])
                ps_rep = psx.tile([128, TC], f32, tag="rep")
                nc.tensor.matmul(ps_rep[:], onesr_t[:], rstd_r[:],
                                 start=True, stop=True)
                rstdf = pool.tile([128, TC], f32, tag="rstdf")
                nc.scalar.copy(rstdf[:], ps_rep[:])
                for kk in range(NKH):
                    nc.vector.tensor_tensor(xb[:, kk, hsl], xb[:, kk, hsl],
                                            rstdf[:], op=MULT)

            # ================= pass 1: stats + qkv + rope =================
            with tc.tile_pool(name="p1x", bufs=2) as xpool, \
                 tc.tile_pool(name="p1w", bufs=2) as wpool, \
                 tc.tile_pool(name="p1c", bufs=1) as c1pool, \
                 tc.tile_pool(name="p1s", bufs=1) as sp, \
                 tc.tile_pool(name="p1e", bufs=3) as xep, \
                 tc.tile_pool(name="p1f", bufs=2) as fp, \
                 tc.tile_pool(name="p1r", bufs=2) as rp, \
                 tc.tile_pool(name="p1t", bufs=1) as tp1, \
                 tc.tile_pool(name="p1o", bufs=4) as op, \
                 tc.tile_pool(name="p1cs", bufs=2) as csp, \
                 tc.tile_pool(name="p1ps", bufs=3, space="PSUM") as psm, \
                 tc.tile_pool(name="p1pr", bufs=1, space="PSUM") as psr1, \
                 tc.tile_pool(name="p1px", bufs=2, space="PSUM") as psx:
                eqkv_t = c1pool.tile([2, NMQ * 128], f32r, tag="eqkv")
                nc.sync.dma_start(eqkv_t[:], eqkv[:])
                for ch in range(NCH):
                    xb = xpool.tile([128, NKH, TC], f32r, tag="xb1")
                    for kp in range(4):
                        nc.sync.dma_start(
                            xb[:, kp * 8:(kp + 1) * 8, :],
                            xd[:, kp * 8:(kp + 1) * 8, ch * TC:(ch + 1) * TC])
                    hsl = slice(0, TC)
                    ps_sum = psx.tile([1, TC], f32, tag="st_sum")
                    ps_sq = psx.tile([1, TC], f32, tag="st_sq")
                    for kk in range(NKH):
                        sq = rp.tile([128, TC], f32r, tag="sq")
                        nc.vector.tensor_tensor(sq[:], xb[:, kk, hsl],
                                                xb[:, kk, hsl], op=MULT)
                        nc.tensor.matmul(ps_sum[:], onesc_t[:], xb[:, kk, hsl],
                                         start=(kk == 0), stop=(kk == NKH - 1))
                        nc.tensor.matmul(ps_sq[:], onesc_t[:], sq[:],
                                         start=(kk == 0), stop=(kk == NKH - 1))
                    mean = sp.tile([1, TC], f32, tag="mean")
                    nc.vector.tensor_scalar_mul(mean[:], ps_sum[:], 1.0 / H)
                    var = sp.tile([1, TC], f32, tag="var")
                    nc.vector.tensor_scalar_mul(var[:], ps_sq[:], 1.0 / H)
                    m2 = sp.tile([1, TC], f32, tag="m2")
                    nc.vector.tensor_tensor(m2[:], mean[:], mean[:], op=MULT)
                    nc.vector.tensor_tensor(var[:], var[:], m2[:], op=SUB)
                    nc.vector.tensor_scalar_add(var[:], var[:], EPS)
                    inv = sp.tile([1, TC], f32, tag="inv")
                    nc.vector.reciprocal(inv[:], var[:])
                    rstd = sp.tile([1, TC], f32, tag="rstd")
                    nc.scalar.sqrt(rstd[:], inv[:])
                    s_t = sp.tile([1, TC], f32, tag="s")
                    nc.vector.scalar_tensor_tensor(s_t[:], mean[:], -1.0,
                                                   rstd[:], op0=MULT, op1=MULT)
                    rstd_r = sp.tile([1, TC], f32r, tag="rstd_r")
                    nc.vector.tensor_copy(rstd_r[:], rstd[:])
                    nc.sync.dma_start(statsd[0:1, ch * TC:(ch + 1) * TC],
                                      rstd_r[:])
                    xe = xep.tile([2, TC], f32r, tag="xe")
                    nc.vector.tensor_copy(xe[0:1, :], s_t[:])
                    nc.sync.dma_start(statsd[1:2, ch * TC:(ch + 1) * TC],
                                      xe[0:1, :])
                    ln_chunk_scale(fp, psr1, xb, hsl, ch, xe, rstd_r)
                    b, cc = ch // (NCH // B), ch % (NCH // B)
                    ca = csp.tile([HALF, TC], f32r, tag="cosc")
                    nc.sync.dma_start(ca[:], cosd[:, b, cc * TC:(cc + 1) * TC])
                    sa = csp.tile([HALF, TC], f32r, tag="sinc")
                    nc.sync.dma_start(sa[:], sind[:, b, cc * TC:(cc + 1) * TC])
                    csl = slice(ch * TC, (ch + 1) * TC)
                    for m in range(NMQ):
                        wts = []
                        for piece in (0, 1):
                            wt = wpool.tile([128, NKH * 64], f32r, tag="wq")
                            nc.sync.dma_start(
                                wt[:],
                                wqkv[m][:, piece * NKH * 64:(piece + 1) * NKH * 64])
                            wts.append(wt)
                        pt = psm.tile([128, TC], f32, tag="mm")
                        for kk in range(NKH):
                            wt = wts[kk // 16]
                            ko = (kk % 16) * 128
                            nc.tensor.matmul(pt[:], wt[:, ko:ko + 128],
                                             xb[:, kk, hsl],
                                             start=(kk == 0), stop=False)
                        nc.tensor.matmul(pt[:], eqkv_t[:, m * 128:(m + 1) * 128],
                                         xe[:], start=False, stop=True)
                        ot = op.tile([128, TC], f32r, tag="sp")
                        if m < 2 * HPC:  # q or k: rope on dims 0..63
                            t1 = tp1.tile([HALF, TC], f32, tag="t1")
                            t2 = tp1.tile([HALF, TC], f32, tag="t2")
                            nc.vector.tensor_tensor(t1[:], pt[0:HALF, :],
                                                    ca[:], op=MULT)
                            nc.vector.tensor_tensor(t2[:], pt[HALF:RD, :],
                                                    sa[:], op=MULT)
                            nc.vector.tensor_tensor(ot[0:HALF, :], t1[:],
                                                    t2[:], op=SUB)
                            t3 = tp1.tile([HALF, TC], f32, tag="t3")
                            t4 = tp1.tile([HALF, TC], f32, tag="t4")
                            nc.vector.tensor_tensor(t3[:], pt[HALF:RD, :],
                                                    ca[:], op=MULT)
                            nc.vector.tensor_tensor(t4[:], pt[0:HALF, :],
                                                    sa[:], op=MULT)
                            nc.vector.tensor_tensor(ot[HALF:RD, :], t3[:],
                                                    t4[:], op=ADD)
                            nc.scalar.copy(ot[RD:128, :], pt[RD:128, :])
                            dst = qs if m < HPC else ks
                            nc.sync.dma_start(dst[m % HPC][:, csl], ot[:])
                        else:
                            nc.scalar.copy(ot[:], pt[:])
                            nc.sync.dma_start(vs[m - 2 * HPC][:, csl], ot[:])

            # ================= pass 2: attention =================
            with tc.tile_pool(name="p2a", bufs=2) as ap, \
                 tc.tile_pool(name="p2c", bufs=1) as c2pool, \
                 tc.tile_pool(name="p2e", bufs=4) as ep, \
                 tc.tile_pool(name="p2s", bufs=2) as sp2, \
                 tc.tile_pool(name="p2o", bufs=2) as op2, \
                 tc.tile_pool(name="p2st", bufs=2, space="PSUM") as pss, \
                 tc.tile_pool(name="p2pa", bufs=2, space="PSUM") as psa, \
                 tc.tile_pool(name="p2pl", bufs=2, space="PSUM") as psl, \
                 tc.tile_pool(name="p2px", bufs=1, space="PSUM") as psx2:
                ident_t = c2pool.tile([128, 128], f32r, tag="ident")
                nc.sync.dma_start(ident_t[:], identd[:])
                mask_t = c2pool.tile([128, 4, TC], f32, tag="mask")
                nc.sync.dma_start(mask_t[:], mask4[:])
                for b in range(B):
                    for h in range(HPC):
                        qsb = ap.tile([128, S], f32r, tag="qsb")
                        nc.sync.dma_start(qsb[:], qs[h][:, b * S:(b + 1) * S])
                        ksb = ap.tile([128, S], f32r, tag="ksb")
                        nc.sync.dma_start(ksb[:], ks[h][:, b * S:(b + 1) * S])
                        vsb = ap.tile([128, S], f32r, tag="vsb")
                        nc.sync.dma_start(vsb[:], vs[h][:, b * S:(b + 1) * S])
                        vtok = ap.tile([128, NJT, 128], f32r, tag="vtok")
                        for j in range(NJT):
                            ptr = psx2.tile([128, TC], f32r, tag="aux")
                            nc.tensor.transpose(ptr[:, 0:128],
                                                vsb[:, j * 128:(j + 1) * 128],
                                                ident_t[:])
                            nc.scalar.copy(vtok[:, j, :], ptr[:, 0:128])
                        for ic in range(NIC):
                            isl = slice(ic * TC, (ic + 1) * TC)
                            nj = (ic + 1) * JPC
                            pl = psl.tile([1, TC], f32, tag="pl")
                            pa = psa.tile([128, TC], f32, tag="pa")
                            for j in range(nj):
                                st = pss.tile([128, TC], f32, tag="st")
                                nc.tensor.matmul(st[:],
                                                 ksb[:, j * 128:(j + 1) * 128],
                                                 qsb[:, isl],
                                                 start=True, stop=True)
                                if j >= ic * JPC:
                                    nc.vector.tensor_tensor(
                                        st[:], st[:], mask_t[:, j - ic * JPC, :],
                                        op=ADD)
                                pexp = ep.tile([128, TC], f32r, tag="pexp")
                                nc.scalar.activation(pexp[:], st[:], AF.Exp,
                                                     scale=SCALE)
                                nc.tensor.matmul(pl[:], onesc_t[:], pexp[:],
                                                 start=(j == 0), stop=(j == nj - 1))
                                nc.tensor.matmul(pa[:], vtok[:, j, :], pexp[:],
                                                 start=(j == 0), stop=(j == nj - 1))
                            rc = sp2.tile([1, TC], f32, tag="rc")
                            nc.vector.reciprocal(rc[:], pl[:])
                            rcr = sp2.tile([1, TC], f32r, tag="rcr")
                            nc.vector.tensor_copy(rcr[:], rc[:])
                            ps_rep = psx2.tile([128, TC], f32, tag="aux")
                            nc.tensor.matmul(ps_rep[:], onesr_t[:], rcr[:],
                                             start=True, stop=True)
                            rfull = sp2.tile([128, TC], f32, tag="rfull")
                            nc.scalar.copy(rfull[:], ps_rep[:])
                            at = op2.tile([128, TC], f32r, tag="at")
                            nc.vector.tensor_tensor(at[:], pa[:], rfull[:], op=MULT)
                            nc.sync.dma_start(
                                attns[h][:, b * S + ic * TC:b * S + (ic + 1) * TC],
                                at[:])

            # ============ pass 3: fc1+gelu, fc2+dense, output ============
            with tc.tile_pool(name="p3h", bufs=2) as hp, \
                 tc.tile_pool(name="p3x", bufs=1) as xp3, \
                 tc.tile_pool(name="p3w", bufs=2) as wp3, \
                 tc.tile_pool(name="p3c", bufs=1) as c3pool, \
                 tc.tile_pool(name="p3a", bufs=2) as ap3, \
                 tc.tile_pool(name="p3s", bufs=2) as sp3, \
                 tc.tile_pool(name="p3o", bufs=2) as op3, \
                 tc.tile_pool(name="p3ps", bufs=3, space="PSUM") as psm3, \
                 tc.tile_pool(name="p3px", bufs=1, space="PSUM") as psx3:
                efc1_t = c3pool.tile([2, NMF1 * 128], f32r, tag="efc1")
                nc.sync.dma_start(efc1_t[:], efc1[:])
                for ch in range(NCH):
                    xh = xp3.tile([128, NKH, TC], f32r, tag="xb3")
                    for kp in range(4):
                        nc.sync.dma_start(
                            xh[:, kp * 8:(kp + 1) * 8, :],
                            xd[:, kp * 8:(kp + 1) * 8, ch * TC:(ch + 1) * TC])
                    rstd_r = sp3.tile([1, TC], f32r, tag="rstd_r")
                    nc.sync.dma_start(rstd_r[:],
                                      statsd[0:1, ch * TC:(ch + 1) * TC])
                    xe = sp3.tile([2, TC], f32r, tag="xe")
                    nc.sync.dma_start(xe[0:1, :],
                                      statsd[1:2, ch * TC:(ch + 1) * TC])
                    ln_chunk_scale(sp3, psx3, xh, slice(0, TC), ch, xe, rstd_r)
                    hb = hp.tile([128, NMF1, TC], f32r, tag="hb")
                    for m in range(NMF1):
                        wts = []
                        for piece in (0, 1):
                            wt = wp3.tile([128, NKH * 64], f32r, tag="wf1")
                            nc.sync.dma_start(
                                wt[:],
                                wfc1[m][:, piece * NKH * 64:(piece + 1) * NKH * 64])
                            wts.append(wt)
                        pt = psm3.tile([128, TC], f32, tag="mm")
                        for kk in range(NKH):
                            wt = wts[kk // 16]
                            ko = (kk % 16) * 128
                            nc.tensor.matmul(pt[:], wt[:, ko:ko + 128],
                                             xh[:, kk, :],
                                             start=(kk == 0), stop=False)
                        nc.tensor.matmul(pt[:],
                                         efc1_t[:, m * 128:(m + 1) * 128],
                                         xe[:], start=False, stop=True)
                        nc.scalar.activation(hb[:, m, :], pt[:], AF.Gelu)
                    atp = ap3.tile([128, HPC, TC], f32r, tag="atp")
                    for h in range(HPC):
                        nc.sync.dma_start(
                            atp[:, h, :],
                            attns[h][:, ch * TC:(ch + 1) * TC])
                    for m in range(NMO):
                        wt2 = wp3.tile([128, NKF2 * 128], f32r, tag="wf2")
                        nc.sync.dma_start(wt2[:], wfc2[m])
                        wtd = wp3.tile([128, HPC * 128], f32r, tag="wd")
                        nc.sync.dma_start(wtd[:], wdns[m])
                        pt = psm3.tile([128, TC], f32, tag="mm")
                        for kk in range(NKF2):
                            nc.tensor.matmul(pt[:],
                                             wt2[:, kk * 128:(kk + 1) * 128],
                                             hb[:, kk, :],
                                             start=(kk == 0), stop=False)
                        for kd in range(HPC):
                            nc.tensor.matmul(pt[:],
                                             wtd[:, kd * 128:(kd + 1) * 128],
                                             atp[:, kd, :],
                                             start=False, stop=(kd == HPC - 1))
                        ot = op3.tile([128, TC], f32, tag="ot")
                        nc.scalar.copy(ot[:], pt[:])
                        nc.sync.dma_start(
                            outd[:, m, ch * TC:(ch + 1) * TC],
                            ot[:])

    nc.compile()
    return nc


def _tile_w(w):
    """[K, M] -> [M//128, 128, K]: [m][p][kk*128+f] = w[kk*128+p, m*128+f]."""
    K, M = w.shape
    nk, nm = K // 128, M // 128
    return np.ascontiguousarray(
        w.reshape(nk, 128, nm, 128).transpose(2, 1, 0, 3).reshape(nm, 128, nk * 128))


def _prep_inputs(position_ids, hidden_states, ln_w, ln_b, qkv_w, qkv_b,
                 fc1_w, fc1_b, fc2_w, dense_w):
    x = np.asarray(hidden_states, np.float32).reshape(T, H)
    xt = np.ascontiguousarray(x.T.reshape(NKH, 128, T).transpose(1, 0, 2))

    # mimic the reference's float32 rope math
    pos = np.asarray(position_ids).astype(np.float32)  # [B, S]
    inv = (1.0 / (np.float32(ROPE_BASE) **
                  (np.arange(0, RD, 2, dtype=np.float32) / np.float32(RD))))
    fr = (pos[:, None, :] * inv[None, :, None]).astype(np.float32)  # [B, 32, S]
    cos = np.cos(fr).astype(np.float32).transpose(1, 0, 2).copy()   # [32, B, S]
    sin = np.sin(fr).astype(np.float32).transpose(1, 0, 2).copy()

    jj = np.arange(128)[:, None]
    ff = np.arange(TC)[None, :]
    mask = np.stack([np.where(a * 128 + jj <= ff, 0.0, MASKV).astype(np.float32)
                     for a in range(4)], axis=1)  # [128, 4, TC]

    ln_w = np.asarray(ln_w, np.float32)
    ln_b = np.asarray(ln_b, np.float32)
    qkv_w = np.asarray(qkv_w, np.float32)
    qkv_b = np.asarray(qkv_b, np.float32)
    fc1_w = np.asarray(fc1_w, np.float32)
    fc1_b = np.asarray(fc1_b, np.float32)
    fc2_w = np.asarray(fc2_w, np.float32)
    dense_w = np.asarray(dense_w, np.float32)

    wq_all = ln_w[:, None] * qkv_w        # [H, 3H]
    c1q_all = qkv_w.T @ ln_w              # [3H]
    cq_all = qkv_w.T @ ln_b + qkv_b       # [3H]
    wf_all = ln_w[:, None] * fc1_w
    c1f_all = fc1_w.T @ ln_w
    cf_all = fc1_w.T @ ln_b + fc1_b

    in_maps = []
    for c in range(8):
        hsel = np.arange(HPC * c * HD, HPC * (c + 1) * HD)
        cols = np.concatenate([hsel, H + hsel, 2 * H + hsel])
        f1sel = np.arange(c * NMF1 * 128, (c + 1) * NMF1 * 128)
        in_maps.append({
            "x": xt,
            "wqkv": _tile_w(np.ascontiguousarray(wq_all[:, cols])),
            "eqkv": np.ascontiguousarray(
                np.stack([c1q_all[cols], cq_all[cols]])).astype(np.float32),
            "wfc1": _tile_w(np.ascontiguousarray(wf_all[:, f1sel])),
            "efc1": np.ascontiguousarray(
                np.stack([c1f_all[f1sel], cf_all[f1sel]])).astype(np.float32),
            "wfc2": _tile_w(np.ascontiguousarray(fc2_w[f1sel, :])),
            "wdns": _tile_w(np.ascontiguousarray(dense_w[hsel, :])),
            "cos": cos, "sin": sin, "mask4": mask,
            "ident": np.eye(128, dtype=np.float32),
            "onesc": np.ones((128, 1), np.float32),
            "onesr": np.ones((1, 128), np.float32),
            "ones512": np.ones((1, TC), np.float32),
        })
    return in_maps


def run(inputs, trace=False):
    """Compile (cached), run on 8 cores, gather. Returns (out, exec_time_ns)."""
    if "nc" not in _cache:
        _cache["nc"] = _build_program()
    nc = _cache["nc"]

    in_maps = _prep_inputs(
        inputs["position_ids"], inputs["hidden_states"], inputs["ln_w"],
        inputs["ln_b"], inputs["qkv_w"], inputs["qkv_b"], inputs["fc1_w"],
        inputs["fc1_b"], inputs["fc2_w"], inputs["dense_w"])

    res = run_bass_kernel_spmd(nc, in_maps, core_ids=list(range(8)), trace=trace)

    acc = res.results[0]["out"].astype(np.float32)
    for c in range(1, 8):
        acc = acc + res.results[c]["out"]
    full_t = acc.transpose(1, 0, 2).reshape(H, T)          # [H, tokens]
    out = np.ascontiguousarray(full_t.T).reshape(B, S, H)
    out = out + np.asarray(inputs["dense_b"], np.float32)
    out = out + np.asarray(inputs["fc2_b"], np.float32)
    out = out + np.asarray(inputs["hidden_states"], np.float32).reshape(B, S, H)
    return out.astype(np.float32), res.exec_time_ns


def kernel(**inputs):
    out, _ = run(inputs, trace=False)
    return out



# revision 27
# speedup vs baseline: 1.4208x; 1.4208x over previous
"""Trainium2 Bass kernel for nn_DecoderLayer_45174466020042 (B=2, S=2048, H=4096).

Tensor-parallel decoder layer on 8 NeuronCores: core c owns heads 4c..4c+4 and
the matching fc1/fc2 column/row slices. v2 design:

- All GEMMs in bf16 (same 1 cycle/row PE rate as f32r, half the DMA/SBUF),
  fp32 PSUM accumulation. Mostly-resident weights per pass (loaded once or
  near-once instead of once per token chunk), cutting HBM traffic ~4x.
- LayerNorm is NOT folded into the activations: matmuls run on raw x and the
  exact correction rides along as a 2-row extra contraction
  [-mu; 1/rstd] x [colsum(W); c0], followed by a per-token *rstd post-scale
  (rope commutes with per-token scaling, so the scale folds into the rope
  cos/sin tiles). This removes the stats->matmul serialization; stats for
  chunk ch+1 are software-pipelined into the middle of chunk ch's matmuls.
- Rope is packed across the 4 heads (all x1 rows in one 128-partition tile,
  all x2 rows in another) so the rotary vector ops run at full DVE width.
- Four passes: p1 stats+qkv+rope, p2 attention (diagonal j-tiles first so
  mask+exp latency hides behind full tiles), p3a fc1+gelu, p3b fc2+dense.
"""
import sys

sys.path.insert(0, '/opt/trn_rl_repo')

import numpy as np
import ml_dtypes
import concourse.bass as bass
import concourse.bacc as bacc
import concourse.tile as tile
from concourse import mybir
from concourse.bass_utils import run_bass_kernel_spmd

bf16 = mybir.dt.bfloat16
f32r = mybir.dt.float32r
f32 = mybir.dt.float32
MULT = mybir.AluOpType.mult
ADD = mybir.AluOpType.add
SUB = mybir.AluOpType.subtract
AF = mybir.ActivationFunctionType
npbf16 = ml_dtypes.bfloat16

B, S, H = 2, 2048, 4096
NH, HD = 32, 128
RD, HALF = 64, 32
EPS = 1e-5
SCALE = HD ** -0.5
ROPE_BASE = 10000.0
T = B * S                 # 4096 tokens
NKH = H // 128            # 32 k-tiles over H
TC = 512                  # token chunk
NCH = T // TC             # 8 chunks
HPC = NH // 8             # 4 heads per core
NMQ = 3 * HPC             # 12 qkv m-tiles per core
NMF1 = 4 * H // 8 // 128  # 16 fc1 m-tiles per core
NMO = H // 128            # 32 output m-tiles
NKF2 = NMF1               # 16 fc2 k-tiles per core
NJT = S // 128            # 16 j-tiles per (b, h)
NIC = S // TC             # 4 i-chunks per (b, h)
JPC = TC // 128           # 4 j-tiles per i-chunk width
MASKV = -600.0            # additive pre-scale mask; exp(MASKV*SCALE) ~ 1e-23
W1R = 10                  # fc1 m-tiles resident in SBUF (rest streamed)
W2R = 24                  # fc2 m-tiles resident in SBUF (rest streamed)

_cache = {}


def _build_program(debug_outputs=False):
    nc = bacc.Bacc("TRN2", target_bir_lowering=False, debug=False)
    dbg = "ExternalOutput" if debug_outputs else "Internal"

    xd = nc.dram_tensor("x", [128, NKH, T], bf16, kind="ExternalInput")
    wq_d = nc.dram_tensor("wq", [NMQ, 128, NKH * 128], bf16, kind="ExternalInput")
    eq_d = nc.dram_tensor("eq", [2, NMQ * 128], bf16, kind="ExternalInput")
    w1_d = nc.dram_tensor("w1", [NMF1, 128, NKH * 128], bf16, kind="ExternalInput")
    e1_d = nc.dram_tensor("e1", [2, NMF1 * 128], bf16, kind="ExternalInput")
    w2_d = nc.dram_tensor("w2", [NMO, 128, NKF2 * 128], bf16, kind="ExternalInput")
    wd_d = nc.dram_tensor("wd", [NMO, 128, HPC * 128], bf16, kind="ExternalInput")
    cosd = nc.dram_tensor("cosr", [128, T], bf16, kind="ExternalInput")
    sind = nc.dram_tensor("sinr", [128, T], bf16, kind="ExternalInput")
    mask4 = nc.dram_tensor("mask4", [128, 4, TC], f32, kind="ExternalInput")
    identd = nc.dram_tensor("ident", [128, 128], bf16, kind="ExternalInput")
    onescbd = nc.dram_tensor("onescb", [128, 1], bf16, kind="ExternalInput")
    ones2d = nc.dram_tensor("ones2", [2, 128], bf16, kind="ExternalInput")
    outd = nc.dram_tensor("out", [128, NMO, T], bf16, kind="ExternalOutput")

    # internal DRAM spills
    qsd = nc.dram_tensor("qs", [4, 128, T], bf16, kind=dbg)  # x1rot x2rot pass01 pass23
    ksd = nc.dram_tensor("ks", [4, 128, T], bf16, kind=dbg)
    vsd = nc.dram_tensor("vs", [HPC, 128, T], bf16, kind=dbg)  # head-major
    attnd = nc.dram_tensor("attns", [HPC, 128, T], bf16, kind=dbg)
    hd = nc.dram_tensor("hmid", [128, NKF2, T], bf16, kind=dbg)
    rstd_d = nc.dram_tensor("rstd_d", [2, T], bf16, kind=dbg)
    xed = nc.dram_tensor("xed", [2, T], bf16, kind=dbg)

    with tile.TileContext(nc) as tc:
        with tc.tile_pool(name="gl", bufs=1) as gl:
            onescb_t = gl.tile([128, 1], bf16, tag="onescb")
            nc.sync.dma_start(onescb_t[:], onescbd[:])
            ones2_t = gl.tile([2, 128], bf16, tag="ones2")
            nc.sync.dma_start(ones2_t[:], ones2d[:])

            def dma_x(ch, pool, tagn):
                xb = pool.tile([128, NKH, TC], bf16, tag=tagn)
                for kp in range(2):
                    nc.sync.dma_start(
                        xb[:, kp * 16:(kp + 1) * 16, :],
                        xd[:, kp * 16:(kp + 1) * 16, ch * TC:(ch + 1) * TC])
                return xb

            # ============ pass 1: stats + qkv + rope (wq resident) ============
            with tc.tile_pool(name="p1w", bufs=1) as wp, \
                 tc.tile_pool(name="p1x", bufs=2) as xp, \
                 tc.tile_pool(name="p1cs", bufs=2) as csp, \
                 tc.tile_pool(name="p1sq", bufs=2) as sqp, \
                 tc.tile_pool(name="p1sm", bufs=1) as smp, \
                 tc.tile_pool(name="p1xe", bufs=2) as xep, \
                 tc.tile_pool(name="p1rf", bufs=1) as rfp, \
                 tc.tile_pool(name="p1tmp", bufs=1) as tmp, \
                 tc.tile_pool(name="p1o", bufs=3) as op, \
                 tc.tile_pool(name="p1st", bufs=1, space="PSUM") as psst, \
                 tc.tile_pool(name="p1mm", bufs=4, space="PSUM") as psmm, \
                 tc.tile_pool(name="p1rep", bufs=1, space="PSUM") as psrep:
                xtiles = {}
                wq_t = wp.tile([128, NMQ, NKH * 128], bf16, tag="wq")
                eq_t = wp.tile([2, NMQ * 128], bf16, tag="eq")

                def emit_stats(ch):
                    """sum/sumsq matmuls + vector finish for chunk ch.
                    Returns (rstd_r f32r [1,TC], xe bf16 [2,TC])."""
                    xb = xtiles[ch]
                    ps_sum = psst.tile([1, TC], f32, tag="ssum")
                    ps_sq = psst.tile([1, TC], f32, tag="ssq")
                    for kk in range(NKH):
                        sqt = sqp.tile([128, TC], bf16, tag="sq", bufs=4)
                        nc.scalar.square(sqt[:], xb[:, kk, :])
                        nc.tensor.matmul(ps_sum[:], onescb_t[:], xb[:, kk, :],
                                         start=(kk == 0), stop=(kk == NKH - 1))
                        nc.tensor.matmul(ps_sq[:], onescb_t[:], sqt[:],
                                         start=(kk == 0), stop=(kk == NKH - 1))
                    mean = smp.tile([1, TC], f32, tag="mean")
                    nc.vector.tensor_scalar_mul(mean[:], ps_sum[:], 1.0 / H)
                    ex2 = smp.tile([1, TC], f32, tag="ex2")
                    nc.vector.tensor_scalar_mul(ex2[:], ps_sq[:], 1.0 / H)
                    m2 = smp.tile([1, TC], f32, tag="m2")
                    nc.vector.tensor_tensor(m2[:], mean[:], mean[:], op=MULT)
                    # ex2 <- (ex2 + EPS) - mean^2  (in place)
                    nc.vector.scalar_tensor_tensor(ex2[:], ex2[:], EPS, m2[:],
                                                   op0=ADD, op1=SUB)
                    inv = smp.tile([1, TC], f32, tag="inv")
                    nc.vector.reciprocal_approx_fast(inv[:], ex2[:])
                    rstdh = smp.tile([1, TC], f32, tag="rstdh")
                    nc.scalar.sqrt(rstdh[:], inv[:])
                    # hi/lo bf16 split of rstd: rep matmul sums the two rows
                    # in fp32 PSUM for ~16-bit precision with a pure-bf16
                    # instruction stream (mixing f32r matmuls into the bf16
                    # stream corrupts alternate columns on HW)
                    rhl = smp.tile([2, TC], bf16, tag="rhl", bufs=2)
                    nc.vector.tensor_copy(rhl[0:1, :], rstdh[:])
                    rlo = smp.tile([1, TC], bf16, tag="rlo", bufs=2)
                    nc.vector.tensor_tensor(rlo[:], rstdh[:], rhl[0:1, :],
                                            op=SUB)
                    nc.sync.dma_start(rhl[1:2, :], rlo[:])
                    invr = smp.tile([1, TC], bf16, tag="invr", bufs=2)
                    nc.scalar.sqrt(invr[:], ex2[:])
                    xe = xep.tile([2, TC], bf16, tag="xe")
                    nc.vector.tensor_scalar_mul(xe[0:1, :], mean[:], -1.0)
                    nc.sync.dma_start(xe[1:2, :], invr[:])
                    nc.sync.dma_start(rstd_d[:, ch * TC:(ch + 1) * TC], rhl[:])
                    nc.sync.dma_start(xed[:, ch * TC:(ch + 1) * TC], xe[:])
                    return rhl, xe

                # prologue: x(0) on the sync queue BEFORE the 12MB wq load
                # (which goes on the scalar queue) so stats(0) starts early;
                # wq m0/m1 issue first so qkv(0) can begin promptly
                xtiles[0] = dma_x(0, xp, "xb")
                for m in range(2):
                    nc.scalar.dma_start(wq_t[:, m, :], wq_d[m])
                nc.scalar.dma_start(eq_t[:], eq_d[:])
                stats = {0: emit_stats(0)}
                for m in range(2, NMQ):
                    nc.scalar.dma_start(wq_t[:, m, :], wq_d[m])

                for ch in range(NCH):
                    csl = slice(ch * TC, (ch + 1) * TC)
                    if ch + 1 < NCH:
                        xtiles[ch + 1] = dma_x(ch + 1, xp, "xb")
                    xb = xtiles[ch]
                    rhl, xe = stats[ch]
                    # broadcast rstd over partitions; fold into cos/sin
                    pr = psrep.tile([128, TC], f32, tag="rep")
                    nc.tensor.matmul(pr[:], ones2_t[:], rhl[:],
                                     start=True, stop=True)
                    rstdf = rfp.tile([128, TC], f32, tag="rstdf")
                    nc.scalar.copy(rstdf[:], pr[:])
                    cosc = csp.tile([128, TC], bf16, tag="cosc")
                    nc.sync.dma_start(cosc[:], cosd[:, csl])
                    sinc = csp.tile([128, TC], bf16, tag="sinc")
                    nc.sync.dma_start(sinc[:], sind[:, csl])
                    cs = rfp.tile([128, TC], f32, tag="cs")
                    nc.vector.tensor_tensor(cs[:], cosc[:], rstdf[:], op=MULT)
                    ss = rfp.tile([128, TC], f32, tag="ss")
                    nc.vector.tensor_tensor(ss[:], sinc[:], rstdf[:], op=MULT)

                    def qkv_mm(m):
                        pt = psmm.tile([128, TC], f32, tag="mm")
                        for kk in range(NKH):
                            nc.tensor.matmul(pt[:],
                                             wq_t[:, m, kk * 128:(kk + 1) * 128],
                                             xb[:, kk, :],
                                             start=(kk == 0), stop=False)
                        nc.tensor.matmul(pt[:], eq_t[:, m * 128:(m + 1) * 128],
                                         xe[:], start=False, stop=True)
                        return pt

                    def rope_pair(pt0, pt1, dst):
                        a = tmp.tile([128, TC], f32, tag="ra")
                        nc.vector.tensor_tensor(a[:], pt0[:], cs[:], op=MULT)
                        b = tmp.tile([128, TC], f32, tag="rb")
                        nc.vector.tensor_tensor(b[:], pt1[:], ss[:], op=MULT)
                        o1 = op.tile([128, TC], bf16, tag="osp")
                        nc.vector.tensor_tensor(o1[:], a[:], b[:], op=SUB)
                        nc.sync.dma_start(dst[0][:, csl], o1[:])
                        c = tmp.tile([128, TC], f32, tag="ra")
                        nc.vector.tensor_tensor(c[:], pt1[:], cs[:], op=MULT)
                        d = tmp.tile([128, TC], f32, tag="rb")
                        nc.vector.tensor_tensor(d[:], pt0[:], ss[:], op=MULT)
                        o2 = op.tile([128, TC], bf16, tag="osp")
                        nc.vector.tensor_tensor(o2[:], c[:], d[:], op=ADD)
                        nc.sync.dma_start(dst[1][:, csl], o2[:])

                    def scale_out(pt, dram_ap):
                        o = op.tile([128, TC], bf16, tag="osp")
                        nc.vector.tensor_tensor(o[:], pt[:], rstdf[:], op=MULT)
                        nc.sync.dma_start(dram_ap, o[:])

                    # q: x1, x2 packed tiles then pass tiles
                    ptq0 = qkv_mm(0)
                    ptq1 = qkv_mm(1)
                    rope_pair(ptq0, ptq1, (qsd[0], qsd[1]))
                    scale_out(qkv_mm(2), qsd[2][:, csl])
                    scale_out(qkv_mm(3), qsd[3][:, csl])
                    # k
                    ptk0 = qkv_mm(4)
                    ptk1 = qkv_mm(5)
                    rope_pair(ptk0, ptk1, (ksd[0], ksd[1]))
                    # pipeline next chunk's stats into the middle of this chunk
                    if ch + 1 < NCH:
                        stats[ch + 1] = emit_stats(ch + 1)
                    scale_out(qkv_mm(6), ksd[2][:, csl])
                    scale_out(qkv_mm(7), ksd[3][:, csl])
                    # v (head-major)
                    for hh in range(HPC):
                        scale_out(qkv_mm(8 + hh), vsd[hh][:, csl])
                    del xtiles[ch]

            # w1 lives across pass 2 (preload) and pass 3a; the x/stat pools
            # for pass 3a are opened alongside so chunk 0's activations can
            # stream in during attention
            with tc.tile_pool(name="w1pool", bufs=1) as w1p, \
                 tc.tile_pool(name="p3x", bufs=2) as xp3, \
                 tc.tile_pool(name="p3sm", bufs=2) as smp3:
                w1_t = w1p.tile([128, W1R, NKH * 128], bf16, tag="w1")
                e1_t = w1p.tile([2, NMF1 * 128], bf16, tag="e1")
                xt3 = {}
                sts3 = {}

                def load_stats3(ch):
                    csl_ = slice(ch * TC, (ch + 1) * TC)
                    rhl3 = smp3.tile([2, TC], bf16, tag="rhl3", name="rhl3")
                    nc.sync.dma_start(rhl3[:], rstd_d[:, csl_])
                    xe = smp3.tile([2, TC], bf16, tag="xe", name="xe3")
                    nc.sync.dma_start(xe[:], xed[:, csl_])
                    return rhl3, xe

                # ================= pass 2: attention =================
                with tc.tile_pool(name="p2c", bufs=1) as c2p, \
                     tc.tile_pool(name="p2g", bufs=2) as gp, \
                     tc.tile_pool(name="p2v", bufs=2) as vp, \
                     tc.tile_pool(name="p2e", bufs=3) as ep, \
                     tc.tile_pool(name="p2s", bufs=1) as sp2, \
                     tc.tile_pool(name="p2o", bufs=2) as op2, \
                     tc.tile_pool(name="p2aux", bufs=2, space="PSUM") as paux, \
                     tc.tile_pool(name="p2st", bufs=3, space="PSUM") as pst, \
                     tc.tile_pool(name="p2pa", bufs=2, space="PSUM") as ppa, \
                     tc.tile_pool(name="p2pl", bufs=1, space="PSUM") as ppl:
                    ident_t = c2p.tile([128, 128], bf16, tag="ident")
                    nc.sync.dma_start(ident_t[:], identd[:])
                    mask_t = c2p.tile([128, 4, TC], f32, tag="mask")
                    nc.sync.dma_start(mask_t[:], mask4[:])
                    # preload resident fc1 weights during attention
                    for m in range(W1R):
                        nc.scalar.dma_start(w1_t[:, m, :], w1_d[m])
                    nc.scalar.dma_start(e1_t[:], e1_d[:])
                    # prefetch pass-3a chunk 0 activations during attention
                    xt3[0] = dma_x(0, xp3, "xb3")
                    sts3[0] = load_stats3(0)

                    def gather(b, h):
                        qsb = gp.tile([128, S], bf16, tag="qsb")
                        ksb = gp.tile([128, S], bf16, tag="ksb")
                        vsb = gp.tile([128, S], bf16, tag="vsb")
                        bsl = slice(b * S, (b + 1) * S)
                        for t, dr in ((qsb, qsd), (ksb, ksd)):
                            nc.sync.dma_start(t[0:32, :],
                                              dr[0][32 * h:32 * h + 32, bsl])
                            nc.sync.dma_start(t[32:64, :],
                                              dr[1][32 * h:32 * h + 32, bsl])
                            nc.sync.dma_start(
                                t[64:128, :],
                                dr[2 + h // 2][64 * (h % 2):64 * (h % 2) + 64,
                                               bsl])
                        nc.sync.dma_start(vsb[:], vsd[h][:, bsl])
                        return qsb, ksb, vsb

                    def transpose_v(vsb):
                        vtok = vp.tile([128, NJT, 128], bf16, tag="vtok")
                        for j in range(NJT):
                            pa_aux = paux.tile([128, 1024], bf16, tag="aux")
                            nc.tensor.transpose(pa_aux[:, 0:128],
                                                vsb[:, j * 128:(j + 1) * 128],
                                                ident_t[:])
                            nc.scalar.copy(vtok[:, j, :], pa_aux[:, 0:128])
                        return vtok

                    bhs = [(b, h) for b in range(B) for h in range(HPC)]
                    gat = {0: gather(*bhs[0])}
                    vtoks = {0: transpose_v(gat[0][2])}
                    for bi, (b, h) in enumerate(bhs):
                        qsb, ksb, vsb = gat[bi]
                        vtok = vtoks[bi]
                        if bi + 1 < len(bhs):
                            gat[bi + 1] = gather(*bhs[bi + 1])
                            vtoks[bi + 1] = transpose_v(gat[bi + 1][2])
                        for ic in range(NIC):
                            isl = slice(ic * TC, (ic + 1) * TC)
                            # diag tile a=0 first (its pa/pl start resets the
                            # full bank); remaining diagonal tiles (mask+exp
                            # latency) are spread between unmasked tiles so
                            # each keeps >=2 steps of exp-pipeline lookahead
                            fulls = list(range(0, ic * JPC))
                            diags = list(range(ic * JPC + 1, ic * JPC + JPC))
                            jlist = [ic * JPC]
                            fi = 0
                            for dj in diags:
                                jlist += fulls[fi:fi + 2]
                                fi += 2
                                jlist.append(dj)
                            jlist += fulls[fi:]
                            nj = len(jlist)
                            pa = ppa.tile([128, TC], f32, tag="pa")
                            pl = ppl.tile([1, TC], f32, tag="pl")

                            def emit_st(idx):
                                j = jlist[idx]
                                a = j - ic * JPC
                                q0 = a * 128 if a > 0 else 0
                                qsl = slice(ic * TC + q0, (ic + 1) * TC)
                                st = pst.tile([128, TC], f32, tag="st")
                                nc.tensor.matmul(st[:, q0:TC],
                                                 ksb[:, j * 128:(j + 1) * 128],
                                                 qsb[:, qsl],
                                                 start=True, stop=True)
                                if a >= 0:
                                    nc.vector.tensor_tensor(
                                        st[:, q0:TC], st[:, q0:TC],
                                        mask_t[:, a, q0:TC], op=ADD)
                                pexp = ep.tile([128, TC], bf16, tag="pexp")
                                nc.scalar.activation(pexp[:, q0:TC],
                                                     st[:, q0:TC], AF.Exp,
                                                     scale=SCALE)
                                return pexp, q0

                            pend = [emit_st(0)]
                            if nj > 1:
                                pend.append(emit_st(1))
                            for idx in range(nj):
                                pexp, q0 = pend.pop(0)
                                j = jlist[idx]
                                if idx + 2 < nj:
                                    pend.append(emit_st(idx + 2))
                                nc.tensor.matmul(pa[:, q0:TC], vtok[:, j, :],
                                                 pexp[:, q0:TC],
                                                 start=(idx == 0),
                                                 stop=(idx == nj - 1))
                                nc.tensor.matmul(pl[:, q0:TC], onescb_t[:],
                                                 pexp[:, q0:TC],
                                                 start=(idx == 0),
                                                 stop=(idx == nj - 1))
                            plsb = sp2.tile([1, TC], f32, tag="plsb")
                            nc.scalar.copy(plsb[:], pl[:])
                            rc = sp2.tile([1, TC], f32, tag="rc")
                            nc.vector.reciprocal_approx_fast(rc[:], plsb[:])
                            rchl = sp2.tile([2, TC], bf16, tag="rchl", bufs=2)
                            nc.vector.tensor_copy(rchl[0:1, :], rc[:])
                            rclo = sp2.tile([1, TC], bf16, tag="rclo", bufs=2)
                            nc.vector.tensor_tensor(rclo[:], rc[:],
                                                    rchl[0:1, :], op=SUB)
                            nc.sync.dma_start(rchl[1:2, :], rclo[:])
                            prr = paux.tile([128, TC], f32, tag="aux")
                            nc.tensor.matmul(prr[:], ones2_t[:], rchl[:],
                                             start=True, stop=True)
                            rcf = sp2.tile([128, TC], f32, tag="rcf")
                            nc.vector.tensor_copy(rcf[:], prr[:])
                            at = op2.tile([128, TC], bf16, tag="at")
                            nc.vector.tensor_tensor(at[:], pa[:], rcf[:],
                                                    op=MULT)
                            nc.sync.dma_start(
                                attnd[h][:,
                                         b * S + ic * TC:b * S + (ic + 1) * TC],
                                at[:])

                # ============ pass 3a: fc1 + gelu (w1 mostly resident) ========
                with tc.tile_pool(name="p3ws", bufs=2) as wsp, \
                     tc.tile_pool(name="p3rf", bufs=1) as rfp3, \
                     tc.tile_pool(name="p3g", bufs=2) as gp3, \
                     tc.tile_pool(name="p3h", bufs=3) as hp3, \
                     tc.tile_pool(name="p3mm", bufs=4, space="PSUM") as psm3, \
                     tc.tile_pool(name="p3rep", bufs=1, space="PSUM") as psr3:
                    for ch in range(NCH):
                        csl = slice(ch * TC, (ch + 1) * TC)
                        if ch + 1 < NCH:
                            xt3[ch + 1] = dma_x(ch + 1, xp3, "xb3")
                            sts3[ch + 1] = load_stats3(ch + 1)
                        xb = xt3[ch]
                        rhl3, xe = sts3[ch]
                        pr = psr3.tile([128, TC], f32, tag="rep")
                        nc.tensor.matmul(pr[:], ones2_t[:], rhl3[:],
                                         start=True, stop=True)
                        rstdf = rfp3.tile([128, TC], f32, tag="rstdf")
                        nc.scalar.copy(rstdf[:], pr[:])
                        # prefetch streamed fc1 weight tiles (m >= W1R)
                        wstiles = {}
                        for m in (W1R, W1R + 1):
                            wstiles[m] = wsp.tile([128, NKH * 128], bf16,
                                                  tag="w1s", name="w1s")
                            nc.sync.dma_start(wstiles[m][:], w1_d[m])
                        for m in range(NMF1):
                            wt = (w1_t[:, m, :] if m < W1R
                                  else wstiles[m][:])
                            pt = psm3.tile([128, TC], f32, tag="mm")
                            for kk in range(NKH):
                                nc.tensor.matmul(pt[:],
                                                 wt[:, kk * 128:(kk + 1) * 128],
                                                 xb[:, kk, :],
                                                 start=(kk == 0), stop=False)
                            nc.tensor.matmul(pt[:],
                                             e1_t[:, m * 128:(m + 1) * 128],
                                             xe[:], start=False, stop=True)
                            if m >= W1R and m + 2 < NMF1:
                                wstiles[m + 2] = wsp.tile(
                                    [128, NKH * 128], bf16, tag="w1s",
                                    name="w1s")
                                nc.sync.dma_start(wstiles[m + 2][:],
                                                  w1_d[m + 2])
                            g = gp3.tile([128, TC], f32, tag="g")
                            nc.vector.tensor_tensor(g[:], pt[:], rstdf[:],
                                                    op=MULT)
                            hb = hp3.tile([128, TC], bf16, tag="hb")
                            nc.scalar.activation(hb[:], g[:], AF.Gelu)
                            nc.sync.dma_start(hd[:, m, csl], hb[:])
                        del xt3[ch]

            # ============ pass 3b: fc2 + dense -> out ============
            with tc.tile_pool(name="p4w", bufs=1) as wp4, \
                 tc.tile_pool(name="p4ws", bufs=2) as wsp4, \
                 tc.tile_pool(name="p4h", bufs=2) as hp4, \
                 tc.tile_pool(name="p4a", bufs=2) as ap4, \
                 tc.tile_pool(name="p4o", bufs=2) as op4, \
                 tc.tile_pool(name="p4mm", bufs=4, space="PSUM") as psm4:
                w2_t = wp4.tile([128, W2R, NKF2 * 128], bf16, tag="w2")
                wd_t = wp4.tile([128, NMO, HPC * 128], bf16, tag="wd")
                for m in range(W2R):
                    nc.sync.dma_start(w2_t[:, m, :], w2_d[m])
                for m in range(NMO):
                    nc.sync.dma_start(wd_t[:, m, :], wd_d[m])
                def load_hb_ab(ch):
                    csl_ = slice(ch * TC, (ch + 1) * TC)
                    hb = hp4.tile([128, NKF2, TC], bf16, tag="hb4", name="hb")
                    for kp in range(2):
                        nc.sync.dma_start(
                            hb[:, kp * 8:(kp + 1) * 8, :],
                            hd[:, kp * 8:(kp + 1) * 8, csl_])
                    ab = ap4.tile([128, HPC, TC], bf16, tag="ab", name="ab")
                    for hh in range(HPC):
                        nc.sync.dma_start(ab[:, hh, :], attnd[hh][:, csl_])
                    return hb, ab

                hbab = {0: load_hb_ab(0)}
                for ch in range(NCH):
                    csl = slice(ch * TC, (ch + 1) * TC)
                    hb, ab = hbab[ch]
                    if ch + 1 < NCH:
                        hbab[ch + 1] = load_hb_ab(ch + 1)
                    w2s = {}
                    for m in (W2R, W2R + 1):
                        w2s[m] = wsp4.tile([128, NKF2 * 128], bf16, tag="w2s",
                                           name="w2s")
                        nc.sync.dma_start(w2s[m][:], w2_d[m])
                    for m in range(NMO):
                        wt = w2_t[:, m, :] if m < W2R else w2s[m][:]
                        pt = psm4.tile([128, TC], f32, tag="mm")
                        for kk in range(NKF2):
                            nc.tensor.matmul(pt[:],
                                             wt[:, kk * 128:(kk + 1) * 128],
                                             hb[:, kk, :],
                                             start=(kk == 0), stop=False)
                        for kd in range(HPC):
                            nc.tensor.matmul(pt[:],
                                             wd_t[:, m, kd * 128:(kd + 1) * 128],
                                             ab[:, kd, :],
                                             start=False, stop=(kd == HPC - 1))
                        if m >= W2R and m + 2 < NMO:
                            w2s[m + 2] = wsp4.tile([128, NKF2 * 128], bf16,
                                                   tag="w2s", name="w2s")
                            nc.sync.dma_start(w2s[m + 2][:], w2_d[m + 2])
                        ot = op4.tile([128, TC], bf16, tag="ot")
                        nc.scalar.copy(ot[:], pt[:])
                        nc.sync.dma_start(outd[:, m, csl], ot[:])

    nc.compile()
    return nc


def _tile_w(w):
    """[K, M] -> [M//128, 128, K]: [m][p][kk*128+f] = w[kk*128+p, m*128+f]."""
    K, M = w.shape
    nk, nm = K // 128, M // 128
    return np.ascontiguousarray(
        w.reshape(nk, 128, nm, 128).transpose(2, 1, 0, 3).reshape(nm, 128, nk * 128))


def _prep_inputs(position_ids, hidden_states, ln_w, ln_b, qkv_w, qkv_b,
                 fc1_w, fc1_b, fc2_w, dense_w):
    x = np.asarray(hidden_states, np.float32).reshape(T, H)
    xt = np.ascontiguousarray(
        x.T.reshape(NKH, 128, T).transpose(1, 0, 2)).astype(npbf16)

    # fp32 rope tables, packed 4x32 over partitions for the head-packed tiles
    pos = np.asarray(position_ids).astype(np.float32)  # [B, S]
    inv = (1.0 / (np.float32(ROPE_BASE) **
                  (np.arange(0, RD, 2, dtype=np.float32) / np.float32(RD))))
    fr = (pos[:, None, :] * inv[None, :, None]).astype(np.float32)  # [B, 32, S]
    cos32 = np.cos(fr).transpose(1, 0, 2).reshape(HALF, T)
    sin32 = np.sin(fr).transpose(1, 0, 2).reshape(HALF, T)
    cos_rep = np.tile(cos32, (4, 1)).astype(npbf16)
    sin_rep = np.tile(sin32, (4, 1)).astype(npbf16)

    jj = np.arange(128)[:, None]
    ff = np.arange(TC)[None, :]
    mask = np.stack([np.where(a * 128 + jj <= ff, 0.0, MASKV).astype(np.float32)
                     for a in range(4)], axis=1)  # [128, 4, TC]

    ln_w = np.asarray(ln_w, np.float32)
    ln_b = np.asarray(ln_b, np.float32)
    qkv_w = np.asarray(qkv_w, np.float32)
    qkv_b = np.asarray(qkv_b, np.float32)
    fc1_w = np.asarray(fc1_w, np.float32)
    fc1_b = np.asarray(fc1_b, np.float32)
    fc2_w = np.asarray(fc2_w, np.float32)
    dense_w = np.asarray(dense_w, np.float32)

    wq_all = ln_w[:, None] * qkv_w        # [H, 3H]
    cq_all = qkv_w.T @ ln_b + qkv_b       # [3H] constant term c0
    wf_all = ln_w[:, None] * fc1_w
    cf_all = fc1_w.T @ ln_b + fc1_b

    in_maps = []
    for c in range(8):
        heads = [HPC * c + i for i in range(HPC)]
        # rope-packed qkv column order
        cols = []
        for base in (0, H):                       # q block, k block
            cols.append([base + g * HD + r for g in heads for r in range(0, 32)])
            cols.append([base + g * HD + r for g in heads for r in range(32, 64)])
            cols.append([base + g * HD + r for g in heads[0:2]
                         for r in range(64, 128)])
            cols.append([base + g * HD + r for g in heads[2:4]
                         for r in range(64, 128)])
        for g in heads:                           # v block, head-major
            cols.append([2 * H + g * HD + r for r in range(HD)])
        flat = np.array([i for blk in cols for i in blk])

        wq_c = wq_all[:, flat].astype(npbf16)
        c1q = wq_c.astype(np.float32).sum(axis=0)
        eq = np.stack([c1q, cq_all[flat]]).astype(npbf16)

        f1sel = np.arange(c * NMF1 * 128, (c + 1) * NMF1 * 128)
        w1_c = wf_all[:, f1sel].astype(npbf16)
        c1f = w1_c.astype(np.float32).sum(axis=0)
        e1 = np.stack([c1f, cf_all[f1sel]]).astype(npbf16)

        hsel = np.arange(HPC * c * HD, HPC * (c + 1) * HD)
        in_maps.append({
            "x": xt,
            "wq": _tile_w(wq_c),
            "eq": eq,
            "w1": _tile_w(w1_c),
            "e1": e1,
            "w2": _tile_w(fc2_w[f1sel, :].astype(npbf16)),
            "wd": _tile_w(dense_w[hsel, :].astype(npbf16)),
            "cosr": cos_rep, "sinr": sin_rep, "mask4": mask,
            "ident": np.eye(128, dtype=npbf16),
            "onescb": np.ones((128, 1), npbf16),
            "ones2": np.ones((2, 128), npbf16),
        })
    return in_maps


def run(inputs, trace=False):
    """Compile (cached), run on 8 cores, gather. Returns (out, exec_time_ns)."""
    if "nc" not in _cache:
        _cache["nc"] = _build_program()
    nc = _cache["nc"]

    in_maps = _prep_inputs(
        inputs["position_ids"], inputs["hidden_states"], inputs["ln_w"],
        inputs["ln_b"], inputs["qkv_w"], inputs["qkv_b"], inputs["fc1_w"],
        inputs["fc1_b"], inputs["fc2_w"], inputs["dense_w"])

    res = run_bass_kernel_spmd(nc, in_maps, core_ids=list(range(8)), trace=trace)

    acc = res.results[0]["out"].astype(np.float32)
    for c in range(1, 8):
        acc = acc + res.results[c]["out"].astype(np.float32)
    full_t = acc.transpose(1, 0, 2).reshape(H, T)          # [H, tokens]
    out = np.ascontiguousarray(full_t.T).reshape(B, S, H)
    out = out + np.asarray(inputs["dense_b"], np.float32)
    out = out + np.asarray(inputs["fc2_b"], np.float32)
    out = out + np.asarray(inputs["hidden_states"], np.float32).reshape(B, S, H)
    return out.astype(np.float32), res.exec_time_ns


def kernel(**inputs):
    out, _ = run(inputs, trace=False)
    return out


# revision 28
# speedup vs baseline: 1.4242x; 1.0024x over previous
"""Trainium2 Bass kernel for nn_DecoderLayer_45174466020042 (B=2, S=2048, H=4096).

Tensor-parallel decoder layer on 8 NeuronCores: core c owns heads 4c..4c+4 and
the matching fc1/fc2 column/row slices. v2 design:

- All GEMMs in bf16 (same 1 cycle/row PE rate as f32r, half the DMA/SBUF),
  fp32 PSUM accumulation. Mostly-resident weights per pass (loaded once or
  near-once instead of once per token chunk), cutting HBM traffic ~4x.
- LayerNorm is NOT folded into the activations: matmuls run on raw x and the
  exact correction rides along as a 2-row extra contraction
  [-mu; 1/rstd] x [colsum(W); c0], followed by a per-token *rstd post-scale
  (rope commutes with per-token scaling, so the scale folds into the rope
  cos/sin tiles). This removes the stats->matmul serialization; stats for
  chunk ch+1 are software-pipelined into the middle of chunk ch's matmuls.
- Rope is packed across the 4 heads (all x1 rows in one 128-partition tile,
  all x2 rows in another) so the rotary vector ops run at full DVE width.
- Four passes: p1 stats+qkv+rope, p2 attention (diagonal j-tiles first so
  mask+exp latency hides behind full tiles), p3a fc1+gelu, p3b fc2+dense.
"""
import sys

sys.path.insert(0, '/opt/trn_rl_repo')

import numpy as np
import ml_dtypes
import concourse.bass as bass
import concourse.bacc as bacc
import concourse.tile as tile
from concourse import mybir
from concourse.bass_utils import run_bass_kernel_spmd

bf16 = mybir.dt.bfloat16
f32r = mybir.dt.float32r
f32 = mybir.dt.float32
MULT = mybir.AluOpType.mult
ADD = mybir.AluOpType.add
SUB = mybir.AluOpType.subtract
AF = mybir.ActivationFunctionType
npbf16 = ml_dtypes.bfloat16

B, S, H = 2, 2048, 4096
NH, HD = 32, 128
RD, HALF = 64, 32
EPS = 1e-5
SCALE = HD ** -0.5
ROPE_BASE = 10000.0
T = B * S                 # 4096 tokens
NKH = H // 128            # 32 k-tiles over H
TC = 512                  # token chunk
NCH = T // TC             # 8 chunks
HPC = NH // 8             # 4 heads per core
NMQ = 3 * HPC             # 12 qkv m-tiles per core
NMF1 = 4 * H // 8 // 128  # 16 fc1 m-tiles per core
NMO = H // 128            # 32 output m-tiles
NKF2 = NMF1               # 16 fc2 k-tiles per core
NJT = S // 128            # 16 j-tiles per (b, h)
NIC = S // TC             # 4 i-chunks per (b, h)
JPC = TC // 128           # 4 j-tiles per i-chunk width
MASKV = -600.0            # additive pre-scale mask; exp(MASKV*SCALE) ~ 1e-23
W1R = 10                  # fc1 m-tiles resident in SBUF (rest streamed)
W2R = 24                  # fc2 m-tiles resident in SBUF (rest streamed)

_cache = {}


def _build_program(debug_outputs=False):
    nc = bacc.Bacc("TRN2", target_bir_lowering=False, debug=False)
    dbg = "ExternalOutput" if debug_outputs else "Internal"

    xd = nc.dram_tensor("x", [128, NKH, T], bf16, kind="ExternalInput")
    wq_d = nc.dram_tensor("wq", [NMQ, 128, NKH * 128], bf16, kind="ExternalInput")
    eq_d = nc.dram_tensor("eq", [2, NMQ * 128], bf16, kind="ExternalInput")
    w1_d = nc.dram_tensor("w1", [NMF1, 128, NKH * 128], bf16, kind="ExternalInput")
    e1_d = nc.dram_tensor("e1", [2, NMF1 * 128], bf16, kind="ExternalInput")
    w2_d = nc.dram_tensor("w2", [NMO, 128, NKF2 * 128], bf16, kind="ExternalInput")
    wd_d = nc.dram_tensor("wd", [NMO, 128, HPC * 128], bf16, kind="ExternalInput")
    cosd = nc.dram_tensor("cosr", [128, T], bf16, kind="ExternalInput")
    sind = nc.dram_tensor("sinr", [128, T], bf16, kind="ExternalInput")
    mask4 = nc.dram_tensor("mask4", [128, 4, TC], f32, kind="ExternalInput")
    identd = nc.dram_tensor("ident", [128, 128], bf16, kind="ExternalInput")
    onescbd = nc.dram_tensor("onescb", [128, 1], bf16, kind="ExternalInput")
    ones2d = nc.dram_tensor("ones2", [2, 128], bf16, kind="ExternalInput")
    outd = nc.dram_tensor("out", [128, NMO, T], bf16, kind="ExternalOutput")

    # internal DRAM spills
    qsd = nc.dram_tensor("qs", [4, 128, T], bf16, kind=dbg)  # x1rot x2rot pass01 pass23
    ksd = nc.dram_tensor("ks", [4, 128, T], bf16, kind=dbg)
    vsd = nc.dram_tensor("vs", [HPC, 128, T], bf16, kind=dbg)  # head-major
    attnd = nc.dram_tensor("attns", [HPC, 128, T], bf16, kind=dbg)
    hd = nc.dram_tensor("hmid", [128, NKF2, T], bf16, kind=dbg)
    rstd_d = nc.dram_tensor("rstd_d", [2, T], bf16, kind=dbg)
    xed = nc.dram_tensor("xed", [2, T], bf16, kind=dbg)

    with tile.TileContext(nc) as tc:
        with tc.tile_pool(name="gl", bufs=1) as gl:
            onescb_t = gl.tile([128, 1], bf16, tag="onescb")
            nc.sync.dma_start(onescb_t[:], onescbd[:])
            ones2_t = gl.tile([2, 128], bf16, tag="ones2")
            nc.sync.dma_start(ones2_t[:], ones2d[:])

            def dma_x(ch, pool, tagn):
                xb = pool.tile([128, NKH, TC], bf16, tag=tagn)
                for kp in range(2):
                    nc.sync.dma_start(
                        xb[:, kp * 16:(kp + 1) * 16, :],
                        xd[:, kp * 16:(kp + 1) * 16, ch * TC:(ch + 1) * TC])
                return xb

            # ============ pass 1: stats + qkv + rope (wq resident) ============
            with tc.tile_pool(name="p1w", bufs=1) as wp, \
                 tc.tile_pool(name="p1x", bufs=2) as xp, \
                 tc.tile_pool(name="p1cs", bufs=2) as csp, \
                 tc.tile_pool(name="p1sq", bufs=2) as sqp, \
                 tc.tile_pool(name="p1sm", bufs=1) as smp, \
                 tc.tile_pool(name="p1xe", bufs=2) as xep, \
                 tc.tile_pool(name="p1rf", bufs=1) as rfp, \
                 tc.tile_pool(name="p1tmp", bufs=1) as tmp, \
                 tc.tile_pool(name="p1o", bufs=3) as op, \
                 tc.tile_pool(name="p1st", bufs=1, space="PSUM") as psst, \
                 tc.tile_pool(name="p1mm", bufs=4, space="PSUM") as psmm, \
                 tc.tile_pool(name="p1rep", bufs=1, space="PSUM") as psrep:
                xtiles = {}
                wq_t = wp.tile([128, NMQ, NKH * 128], bf16, tag="wq")
                eq_t = wp.tile([2, NMQ * 128], bf16, tag="eq")

                def emit_stats(ch):
                    """sum/sumsq matmuls + vector finish for chunk ch.
                    Returns (rstd_r f32r [1,TC], xe bf16 [2,TC])."""
                    xb = xtiles[ch]
                    ps_sum = psst.tile([1, TC], f32, tag="ssum")
                    ps_sq = psst.tile([1, TC], f32, tag="ssq")
                    for kk in range(NKH):
                        sqt = sqp.tile([128, TC], bf16, tag="sq", bufs=3)
                        nc.scalar.square(sqt[:], xb[:, kk, :])
                        nc.tensor.matmul(ps_sum[:], onescb_t[:], xb[:, kk, :],
                                         start=(kk == 0), stop=(kk == NKH - 1))
                        nc.tensor.matmul(ps_sq[:], onescb_t[:], sqt[:],
                                         start=(kk == 0), stop=(kk == NKH - 1))
                    mean = smp.tile([1, TC], f32, tag="mean")
                    nc.vector.tensor_scalar_mul(mean[:], ps_sum[:], 1.0 / H)
                    ex2 = smp.tile([1, TC], f32, tag="ex2")
                    nc.vector.tensor_scalar_mul(ex2[:], ps_sq[:], 1.0 / H)
                    m2 = smp.tile([1, TC], f32, tag="m2")
                    nc.vector.tensor_tensor(m2[:], mean[:], mean[:], op=MULT)
                    # ex2 <- (ex2 + EPS) - mean^2  (in place)
                    nc.vector.scalar_tensor_tensor(ex2[:], ex2[:], EPS, m2[:],
                                                   op0=ADD, op1=SUB)
                    inv = smp.tile([1, TC], f32, tag="inv")
                    nc.vector.reciprocal_approx_fast(inv[:], ex2[:])
                    rstdh = smp.tile([1, TC], f32, tag="rstdh")
                    nc.scalar.sqrt(rstdh[:], inv[:])
                    # hi/lo bf16 split of rstd: rep matmul sums the two rows
                    # in fp32 PSUM for ~16-bit precision with a pure-bf16
                    # instruction stream (mixing f32r matmuls into the bf16
                    # stream corrupts alternate columns on HW)
                    rhl = smp.tile([2, TC], bf16, tag="rhl", bufs=2)
                    nc.vector.tensor_copy(rhl[0:1, :], rstdh[:])
                    rlo = smp.tile([1, TC], bf16, tag="rlo", bufs=2)
                    nc.vector.tensor_tensor(rlo[:], rstdh[:], rhl[0:1, :],
                                            op=SUB)
                    nc.sync.dma_start(rhl[1:2, :], rlo[:])
                    invr = smp.tile([1, TC], bf16, tag="invr", bufs=2)
                    nc.scalar.sqrt(invr[:], ex2[:])
                    xe = xep.tile([2, TC], bf16, tag="xe")
                    nc.vector.tensor_scalar_mul(xe[0:1, :], mean[:], -1.0)
                    nc.sync.dma_start(xe[1:2, :], invr[:])
                    nc.sync.dma_start(rstd_d[:, ch * TC:(ch + 1) * TC], rhl[:])
                    nc.sync.dma_start(xed[:, ch * TC:(ch + 1) * TC], xe[:])
                    return rhl, xe

                # prologue: x(0) on the sync queue BEFORE the 12MB wq load
                # (which goes on the scalar queue) so stats(0) starts early;
                # wq m0/m1 issue first so qkv(0) can begin promptly
                xtiles[0] = dma_x(0, xp, "xb")
                for m in range(2):
                    nc.scalar.dma_start(wq_t[:, m, :], wq_d[m])
                nc.scalar.dma_start(eq_t[:], eq_d[:])
                stats = {0: emit_stats(0)}
                for m in range(2, NMQ):
                    nc.scalar.dma_start(wq_t[:, m, :], wq_d[m])

                for ch in range(NCH):
                    csl = slice(ch * TC, (ch + 1) * TC)
                    if ch + 1 < NCH:
                        xtiles[ch + 1] = dma_x(ch + 1, xp, "xb")
                    xb = xtiles[ch]
                    rhl, xe = stats[ch]
                    # broadcast rstd over partitions; fold into cos/sin
                    pr = psrep.tile([128, TC], f32, tag="rep")
                    nc.tensor.matmul(pr[:], ones2_t[:], rhl[:],
                                     start=True, stop=True)
                    rstdf = rfp.tile([128, TC], f32, tag="rstdf")
                    nc.scalar.copy(rstdf[:], pr[:])
                    cosc = csp.tile([128, TC], bf16, tag="cosc")
                    nc.sync.dma_start(cosc[:], cosd[:, csl])
                    sinc = csp.tile([128, TC], bf16, tag="sinc")
                    nc.sync.dma_start(sinc[:], sind[:, csl])
                    cs = rfp.tile([128, TC], f32, tag="cs")
                    nc.vector.tensor_tensor(cs[:], cosc[:], rstdf[:], op=MULT)
                    ss = rfp.tile([128, TC], f32, tag="ss")
                    nc.vector.tensor_tensor(ss[:], sinc[:], rstdf[:], op=MULT)

                    def qkv_mm(m):
                        pt = psmm.tile([128, TC], f32, tag="mm")
                        for kk in range(NKH):
                            nc.tensor.matmul(pt[:],
                                             wq_t[:, m, kk * 128:(kk + 1) * 128],
                                             xb[:, kk, :],
                                             start=(kk == 0), stop=False)
                        nc.tensor.matmul(pt[:], eq_t[:, m * 128:(m + 1) * 128],
                                         xe[:], start=False, stop=True)
                        return pt

                    def rope_pair(pt0, pt1, dst):
                        a = tmp.tile([128, TC], f32, tag="ra")
                        nc.vector.tensor_tensor(a[:], pt0[:], cs[:], op=MULT)
                        b = tmp.tile([128, TC], f32, tag="rb")
                        nc.vector.tensor_tensor(b[:], pt1[:], ss[:], op=MULT)
                        o1 = op.tile([128, TC], bf16, tag="osp")
                        nc.vector.tensor_tensor(o1[:], a[:], b[:], op=SUB)
                        nc.sync.dma_start(dst[0][:, csl], o1[:])
                        c = tmp.tile([128, TC], f32, tag="ra")
                        nc.vector.tensor_tensor(c[:], pt1[:], cs[:], op=MULT)
                        d = tmp.tile([128, TC], f32, tag="rb")
                        nc.vector.tensor_tensor(d[:], pt0[:], ss[:], op=MULT)
                        o2 = op.tile([128, TC], bf16, tag="osp")
                        nc.vector.tensor_tensor(o2[:], c[:], d[:], op=ADD)
                        nc.sync.dma_start(dst[1][:, csl], o2[:])

                    def scale_out(pt, dram_ap):
                        o = op.tile([128, TC], bf16, tag="osp")
                        nc.vector.tensor_tensor(o[:], pt[:], rstdf[:], op=MULT)
                        nc.sync.dma_start(dram_ap, o[:])

                    # q: x1, x2 packed tiles then pass tiles
                    ptq0 = qkv_mm(0)
                    ptq1 = qkv_mm(1)
                    rope_pair(ptq0, ptq1, (qsd[0], qsd[1]))
                    scale_out(qkv_mm(2), qsd[2][:, csl])
                    scale_out(qkv_mm(3), qsd[3][:, csl])
                    # k
                    ptk0 = qkv_mm(4)
                    ptk1 = qkv_mm(5)
                    rope_pair(ptk0, ptk1, (ksd[0], ksd[1]))
                    # pipeline next chunk's stats into the middle of this chunk
                    if ch + 1 < NCH:
                        stats[ch + 1] = emit_stats(ch + 1)
                    scale_out(qkv_mm(6), ksd[2][:, csl])
                    scale_out(qkv_mm(7), ksd[3][:, csl])
                    # v (head-major)
                    for hh in range(HPC):
                        scale_out(qkv_mm(8 + hh), vsd[hh][:, csl])
                    del xtiles[ch]

            # w1 lives across pass 2 (preload) and pass 3a; the x/stat pools
            # for pass 3a are opened alongside so chunk 0's activations can
            # stream in during attention
            with tc.tile_pool(name="w1pool", bufs=1) as w1p, \
                 tc.tile_pool(name="p3x", bufs=2) as xp3, \
                 tc.tile_pool(name="p3sm", bufs=2) as smp3:
                w1_t = w1p.tile([128, W1R, NKH * 128], bf16, tag="w1")
                e1_t = w1p.tile([2, NMF1 * 128], bf16, tag="e1")
                xt3 = {}
                sts3 = {}

                def load_stats3(ch):
                    csl_ = slice(ch * TC, (ch + 1) * TC)
                    rhl3 = smp3.tile([2, TC], bf16, tag="rhl3", name="rhl3")
                    nc.sync.dma_start(rhl3[:], rstd_d[:, csl_])
                    xe = smp3.tile([2, TC], bf16, tag="xe", name="xe3")
                    nc.sync.dma_start(xe[:], xed[:, csl_])
                    return rhl3, xe

                # ================= pass 2: attention =================
                with tc.tile_pool(name="p2c", bufs=1) as c2p, \
                     tc.tile_pool(name="p2g", bufs=2) as gp, \
                     tc.tile_pool(name="p2v", bufs=2) as vp, \
                     tc.tile_pool(name="p2e", bufs=3) as ep, \
                     tc.tile_pool(name="p2s", bufs=1) as sp2, \
                     tc.tile_pool(name="p2o", bufs=2) as op2, \
                     tc.tile_pool(name="p2aux", bufs=2, space="PSUM") as paux, \
                     tc.tile_pool(name="p2st", bufs=3, space="PSUM") as pst, \
                     tc.tile_pool(name="p2pa", bufs=2, space="PSUM") as ppa, \
                     tc.tile_pool(name="p2pl", bufs=1, space="PSUM") as ppl:
                    ident_t = c2p.tile([128, 128], bf16, tag="ident")
                    nc.sync.dma_start(ident_t[:], identd[:])
                    mask_t = c2p.tile([128, 4, TC], f32, tag="mask")
                    nc.sync.dma_start(mask_t[:], mask4[:])
                    # preload resident fc1 weights during attention
                    for m in range(W1R):
                        nc.scalar.dma_start(w1_t[:, m, :], w1_d[m])
                    nc.scalar.dma_start(e1_t[:], e1_d[:])
                    # prefetch pass-3a chunk 0 activations during attention
                    xt3[0] = dma_x(0, xp3, "xb3")
                    sts3[0] = load_stats3(0)

                    def gather(b, h):
                        qsb = gp.tile([128, S], bf16, tag="qsb")
                        ksb = gp.tile([128, S], bf16, tag="ksb")
                        vsb = gp.tile([128, S], bf16, tag="vsb")
                        bsl = slice(b * S, (b + 1) * S)
                        for t, dr in ((qsb, qsd), (ksb, ksd)):
                            nc.sync.dma_start(t[0:32, :],
                                              dr[0][32 * h:32 * h + 32, bsl])
                            nc.sync.dma_start(t[32:64, :],
                                              dr[1][32 * h:32 * h + 32, bsl])
                            nc.sync.dma_start(
                                t[64:128, :],
                                dr[2 + h // 2][64 * (h % 2):64 * (h % 2) + 64,
                                               bsl])
                        nc.sync.dma_start(vsb[:], vsd[h][:, bsl])
                        return qsb, ksb, vsb

                    def transpose_v(vsb):
                        vtok = vp.tile([128, NJT, 128], bf16, tag="vtok")
                        for j in range(NJT):
                            pa_aux = paux.tile([128, 1024], bf16, tag="aux")
                            nc.tensor.transpose(pa_aux[:, 0:128],
                                                vsb[:, j * 128:(j + 1) * 128],
                                                ident_t[:])
                            nc.scalar.copy(vtok[:, j, :], pa_aux[:, 0:128])
                        return vtok

                    bhs = [(b, h) for b in range(B) for h in range(HPC)]
                    gat = {0: gather(*bhs[0])}
                    vtoks = {0: transpose_v(gat[0][2])}
                    for bi, (b, h) in enumerate(bhs):
                        qsb, ksb, vsb = gat[bi]
                        vtok = vtoks[bi]
                        if bi + 1 < len(bhs):
                            gat[bi + 1] = gather(*bhs[bi + 1])
                            vtoks[bi + 1] = transpose_v(gat[bi + 1][2])
                        for ic in range(NIC):
                            isl = slice(ic * TC, (ic + 1) * TC)
                            # diag tile a=0 first (its pa/pl start resets the
                            # full bank), then unmasked tiles, then remaining
                            # diagonal tiles on shrinking query suffixes
                            jlist = ([ic * JPC] + list(range(0, ic * JPC))
                                     + list(range(ic * JPC + 1,
                                                  ic * JPC + JPC)))
                            nj = len(jlist)
                            pa = ppa.tile([128, TC], f32, tag="pa")
                            pl = ppl.tile([1, TC], f32, tag="pl")

                            def emit_st(idx):
                                j = jlist[idx]
                                a = j - ic * JPC
                                q0 = a * 128 if a > 0 else 0
                                qsl = slice(ic * TC + q0, (ic + 1) * TC)
                                st = pst.tile([128, TC], f32, tag="st")
                                nc.tensor.matmul(st[:, q0:TC],
                                                 ksb[:, j * 128:(j + 1) * 128],
                                                 qsb[:, qsl],
                                                 start=True, stop=True)
                                if a >= 0:
                                    nc.vector.tensor_tensor(
                                        st[:, q0:TC], st[:, q0:TC],
                                        mask_t[:, a, q0:TC], op=ADD)
                                pexp = ep.tile([128, TC], bf16, tag="pexp")
                                nc.scalar.activation(pexp[:, q0:TC],
                                                     st[:, q0:TC], AF.Exp,
                                                     scale=SCALE)
                                return pexp, q0

                            pend = [emit_st(0)]
                            if nj > 1:
                                pend.append(emit_st(1))
                            for idx in range(nj):
                                pexp, q0 = pend.pop(0)
                                j = jlist[idx]
                                if idx + 2 < nj:
                                    pend.append(emit_st(idx + 2))
                                nc.tensor.matmul(pa[:, q0:TC], vtok[:, j, :],
                                                 pexp[:, q0:TC],
                                                 start=(idx == 0),
                                                 stop=(idx == nj - 1))
                                nc.tensor.matmul(pl[:, q0:TC], onescb_t[:],
                                                 pexp[:, q0:TC],
                                                 start=(idx == 0),
                                                 stop=(idx == nj - 1))
                            plsb = sp2.tile([1, TC], f32, tag="plsb")
                            nc.scalar.copy(plsb[:], pl[:])
                            rc = sp2.tile([1, TC], f32, tag="rc")
                            nc.vector.reciprocal_approx_fast(rc[:], plsb[:])
                            rchl = sp2.tile([2, TC], bf16, tag="rchl", bufs=2)
                            nc.vector.tensor_copy(rchl[0:1, :], rc[:])
                            rclo = sp2.tile([1, TC], bf16, tag="rclo", bufs=2)
                            nc.vector.tensor_tensor(rclo[:], rc[:],
                                                    rchl[0:1, :], op=SUB)
                            nc.sync.dma_start(rchl[1:2, :], rclo[:])
                            prr = paux.tile([128, TC], f32, tag="aux")
                            nc.tensor.matmul(prr[:], ones2_t[:], rchl[:],
                                             start=True, stop=True)
                            rcf = sp2.tile([128, TC], f32, tag="rcf")
                            nc.vector.tensor_copy(rcf[:], prr[:])
                            at = op2.tile([128, TC], bf16, tag="at")
                            nc.vector.tensor_tensor(at[:], pa[:], rcf[:],
                                                    op=MULT)
                            nc.sync.dma_start(
                                attnd[h][:,
                                         b * S + ic * TC:b * S + (ic + 1) * TC],
                                at[:])

                # ============ pass 3a: fc1 + gelu (w1 mostly resident) ========
                with tc.tile_pool(name="p3ws", bufs=2) as wsp, \
                     tc.tile_pool(name="p3rf", bufs=1) as rfp3, \
                     tc.tile_pool(name="p3g", bufs=2) as gp3, \
                     tc.tile_pool(name="p3h", bufs=3) as hp3, \
                     tc.tile_pool(name="p3mm", bufs=4, space="PSUM") as psm3, \
                     tc.tile_pool(name="p3rep", bufs=1, space="PSUM") as psr3:
                    for ch in range(NCH):
                        csl = slice(ch * TC, (ch + 1) * TC)
                        if ch + 1 < NCH:
                            xt3[ch + 1] = dma_x(ch + 1, xp3, "xb3")
                            sts3[ch + 1] = load_stats3(ch + 1)
                        xb = xt3[ch]
                        rhl3, xe = sts3[ch]
                        pr = psr3.tile([128, TC], f32, tag="rep")
                        nc.tensor.matmul(pr[:], ones2_t[:], rhl3[:],
                                         start=True, stop=True)
                        rstdf = rfp3.tile([128, TC], f32, tag="rstdf")
                        nc.scalar.copy(rstdf[:], pr[:])
                        # prefetch streamed fc1 weight tiles (m >= W1R)
                        wstiles = {}
                        for m in (W1R, W1R + 1):
                            wstiles[m] = wsp.tile([128, NKH * 128], bf16,
                                                  tag="w1s", name="w1s")
                            nc.sync.dma_start(wstiles[m][:], w1_d[m])
                        for m in range(NMF1):
                            wt = (w1_t[:, m, :] if m < W1R
                                  else wstiles[m][:])
                            pt = psm3.tile([128, TC], f32, tag="mm")
                            for kk in range(NKH):
                                nc.tensor.matmul(pt[:],
                                                 wt[:, kk * 128:(kk + 1) * 128],
                                                 xb[:, kk, :],
                                                 start=(kk == 0), stop=False)
                            nc.tensor.matmul(pt[:],
                                             e1_t[:, m * 128:(m + 1) * 128],
                                             xe[:], start=False, stop=True)
                            if m >= W1R and m + 2 < NMF1:
                                wstiles[m + 2] = wsp.tile(
                                    [128, NKH * 128], bf16, tag="w1s",
                                    name="w1s")
                                nc.sync.dma_start(wstiles[m + 2][:],
                                                  w1_d[m + 2])
                            g = gp3.tile([128, TC], f32, tag="g")
                            nc.vector.tensor_tensor(g[:], pt[:], rstdf[:],
                                                    op=MULT)
                            hb = hp3.tile([128, TC], bf16, tag="hb")
                            nc.scalar.activation(hb[:], g[:], AF.Gelu)
                            nc.sync.dma_start(hd[:, m, csl], hb[:])
                        del xt3[ch]

            # ============ pass 3b: fc2 + dense -> out ============
            with tc.tile_pool(name="p4w", bufs=1) as wp4, \
                 tc.tile_pool(name="p4ws", bufs=2) as wsp4, \
                 tc.tile_pool(name="p4h", bufs=2) as hp4, \
                 tc.tile_pool(name="p4a", bufs=2) as ap4, \
                 tc.tile_pool(name="p4o", bufs=2) as op4, \
                 tc.tile_pool(name="p4mm", bufs=4, space="PSUM") as psm4:
                w2_t = wp4.tile([128, W2R, NKF2 * 128], bf16, tag="w2")
                wd_t = wp4.tile([128, NMO, HPC * 128], bf16, tag="wd")
                for m in range(W2R):
                    nc.sync.dma_start(w2_t[:, m, :], w2_d[m])
                for m in range(NMO):
                    nc.sync.dma_start(wd_t[:, m, :], wd_d[m])
                def load_hb_ab(ch):
                    csl_ = slice(ch * TC, (ch + 1) * TC)
                    hb = hp4.tile([128, NKF2, TC], bf16, tag="hb4", name="hb")
                    for kp in range(2):
                        nc.sync.dma_start(
                            hb[:, kp * 8:(kp + 1) * 8, :],
                            hd[:, kp * 8:(kp + 1) * 8, csl_])
                    ab = ap4.tile([128, HPC, TC], bf16, tag="ab", name="ab")
                    for hh in range(HPC):
                        nc.sync.dma_start(ab[:, hh, :], attnd[hh][:, csl_])
                    return hb, ab

                hbab = {0: load_hb_ab(0)}
                for ch in range(NCH):
                    csl = slice(ch * TC, (ch + 1) * TC)
                    hb, ab = hbab[ch]
                    if ch + 1 < NCH:
                        hbab[ch + 1] = load_hb_ab(ch + 1)
                    w2s = {}
                    for m in (W2R, W2R + 1):
                        w2s[m] = wsp4.tile([128, NKF2 * 128], bf16, tag="w2s",
                                           name="w2s")
                        nc.sync.dma_start(w2s[m][:], w2_d[m])
                    for m in range(NMO):
                        wt = w2_t[:, m, :] if m < W2R else w2s[m][:]
                        pt = psm4.tile([128, TC], f32, tag="mm")
                        for kk in range(NKF2):
                            nc.tensor.matmul(pt[:],
                                             wt[:, kk * 128:(kk + 1) * 128],
                                             hb[:, kk, :],
                                             start=(kk == 0), stop=False)
                        for kd in range(HPC):
                            nc.tensor.matmul(pt[:],
                                             wd_t[:, m, kd * 128:(kd + 1) * 128],
                                             ab[:, kd, :],
                                             start=False, stop=(kd == HPC - 1))
                        if m >= W2R and m + 2 < NMO:
                            w2s[m + 2] = wsp4.tile([128, NKF2 * 128], bf16,
                                                   tag="w2s", name="w2s")
                            nc.sync.dma_start(w2s[m + 2][:], w2_d[m + 2])
                        ot = op4.tile([128, TC], bf16, tag="ot")
                        nc.scalar.copy(ot[:], pt[:])
                        nc.sync.dma_start(outd[:, m, csl], ot[:])

    nc.compile()
    return nc


def _tile_w(w):
    """[K, M] -> [M//128, 128, K]: [m][p][kk*128+f] = w[kk*128+p, m*128+f]."""
    K, M = w.shape
    nk, nm = K // 128, M // 128
    return np.ascontiguousarray(
        w.reshape(nk, 128, nm, 128).transpose(2, 1, 0, 3).reshape(nm, 128, nk * 128))


def _prep_inputs(position_ids, hidden_states, ln_w, ln_b, qkv_w, qkv_b,
                 fc1_w, fc1_b, fc2_w, dense_w):
    x = np.asarray(hidden_states, np.float32).reshape(T, H)
    xt = np.ascontiguousarray(
        x.T.reshape(NKH, 128, T).transpose(1, 0, 2)).astype(npbf16)

    # fp32 rope tables, packed 4x32 over partitions for the head-packed tiles
    pos = np.asarray(position_ids).astype(np.float32)  # [B, S]
    inv = (1.0 / (np.float32(ROPE_BASE) **
                  (np.arange(0, RD, 2, dtype=np.float32) / np.float32(RD))))
    fr = (pos[:, None, :] * inv[None, :, None]).astype(np.float32)  # [B, 32, S]
    cos32 = np.cos(fr).transpose(1, 0, 2).reshape(HALF, T)
    sin32 = np.sin(fr).transpose(1, 0, 2).reshape(HALF, T)
    cos_rep = np.tile(cos32, (4, 1)).astype(npbf16)
    sin_rep = np.tile(sin32, (4, 1)).astype(npbf16)

    jj = np.arange(128)[:, None]
    ff = np.arange(TC)[None, :]
    mask = np.stack([np.where(a * 128 + jj <= ff, 0.0, MASKV).astype(np.float32)
                     for a in range(4)], axis=1)  # [128, 4, TC]

    ln_w = np.asarray(ln_w, np.float32)
    ln_b = np.asarray(ln_b, np.float32)
    qkv_w = np.asarray(qkv_w, np.float32)
    qkv_b = np.asarray(qkv_b, np.float32)
    fc1_w = np.asarray(fc1_w, np.float32)
    fc1_b = np.asarray(fc1_b, np.float32)
    fc2_w = np.asarray(fc2_w, np.float32)
    dense_w = np.asarray(dense_w, np.float32)

    wq_all = ln_w[:, None] * qkv_w        # [H, 3H]
    cq_all = qkv_w.T @ ln_b + qkv_b       # [3H] constant term c0
    wf_all = ln_w[:, None] * fc1_w
    cf_all = fc1_w.T @ ln_b + fc1_b

    in_maps = []
    for c in range(8):
        heads = [HPC * c + i for i in range(HPC)]
        # rope-packed qkv column order
        cols = []
        for base in (0, H):                       # q block, k block
            cols.append([base + g * HD + r for g in heads for r in range(0, 32)])
            cols.append([base + g * HD + r for g in heads for r in range(32, 64)])
            cols.append([base + g * HD + r for g in heads[0:2]
                         for r in range(64, 128)])
            cols.append([base + g * HD + r for g in heads[2:4]
                         for r in range(64, 128)])
        for g in heads:                           # v block, head-major
            cols.append([2 * H + g * HD + r for r in range(HD)])
        flat = np.array([i for blk in cols for i in blk])

        wq_c = wq_all[:, flat].astype(npbf16)
        c1q = wq_c.astype(np.float32).sum(axis=0)
        eq = np.stack([c1q, cq_all[flat]]).astype(npbf16)

        f1sel = np.arange(c * NMF1 * 128, (c + 1) * NMF1 * 128)
        w1_c = wf_all[:, f1sel].astype(npbf16)
        c1f = w1_c.astype(np.float32).sum(axis=0)
        e1 = np.stack([c1f, cf_all[f1sel]]).astype(npbf16)

        hsel = np.arange(HPC * c * HD, HPC * (c + 1) * HD)
        in_maps.append({
            "x": xt,
            "wq": _tile_w(wq_c),
            "eq": eq,
            "w1": _tile_w(w1_c),
            "e1": e1,
            "w2": _tile_w(fc2_w[f1sel, :].astype(npbf16)),
            "wd": _tile_w(dense_w[hsel, :].astype(npbf16)),
            "cosr": cos_rep, "sinr": sin_rep, "mask4": mask,
            "ident": np.eye(128, dtype=npbf16),
            "onescb": np.ones((128, 1), npbf16),
            "ones2": np.ones((2, 128), npbf16),
        })
    return in_maps


def run(inputs, trace=False):
    """Compile (cached), run on 8 cores, gather. Returns (out, exec_time_ns)."""
    if "nc" not in _cache:
        _cache["nc"] = _build_program()
    nc = _cache["nc"]

    in_maps = _prep_inputs(
        inputs["position_ids"], inputs["hidden_states"], inputs["ln_w"],
        inputs["ln_b"], inputs["qkv_w"], inputs["qkv_b"], inputs["fc1_w"],
        inputs["fc1_b"], inputs["fc2_w"], inputs["dense_w"])

    res = run_bass_kernel_spmd(nc, in_maps, core_ids=list(range(8)), trace=trace)

    acc = res.results[0]["out"].astype(np.float32)
    for c in range(1, 8):
        acc = acc + res.results[c]["out"].astype(np.float32)
    full_t = acc.transpose(1, 0, 2).reshape(H, T)          # [H, tokens]
    out = np.ascontiguousarray(full_t.T).reshape(B, S, H)
    out = out + np.asarray(inputs["dense_b"], np.float32)
    out = out + np.asarray(inputs["fc2_b"], np.float32)
    out = out + np.asarray(inputs["hidden_states"], np.float32).reshape(B, S, H)
    return out.astype(np.float32), res.exec_time_ns


def kernel(**inputs):
    out, _ = run(inputs, trace=False)
    return out
